# revision 3
# baseline (speedup 1.0000x reference)
"""Fp8 per-token/per-channel quantized linear for Trainium2, 8 NeuronCores.

Computation (matches the jax reference):
    amax[m]  = max_k |x[m, k]|                       (x is bf16)
    xs[m]    = max(amax, 1e-10) / 448
    x_q      = e4m3fn_round(x / xs)                  (values up to +-448)
    out      = bf16((x_q @ W^T) * xs * w_scales) + bf16(bias)

Mapping to TRN2 hardware:
  * TRN's fp8 E4M3 saturates at +-240 (256..448 are Inf/NaN), so we quantize
    at HALF scale: x_q' = e4m3_round(x * inv) with inv = bf16(224/amax), and
    dequantize with 1/inv.  The reference weights are already exactly
    fp8-representable, so casting them to fp8 is lossless.
  * Sharding: row-parallel over M (8 cores x 1024 rows).  Each core quantizes
    only its own rows and streams the full weight (fp8, host-transposed).
  * Quantization is FUSED into the on-chip transpose: the PE transposes each
    bf16 x tile with rhs = diag(inv) (so the matmul computes x^T * inv), and
    the PSUM->SBUF eviction casts f32->fp8.  No separate quant pass.
  * Loop order interleaves tile t+1's transposes between tile t's GEMM
    blocks so the in-order PE queue never waits on the quant pipeline.
  * Main GEMM runs in fp8 with perf_mode=DoubleRow (k=256 per matmul).
"""

import os
import numpy as np
import ml_dtypes
from contextlib import ExitStack

import concourse.bass as bass
import concourse.bacc as bacc
import concourse.tile as tile
from concourse import mybir
from concourse.bass_utils import run_bass_kernel_spmd
from concourse.masks import make_identity

P = 128
M, K, N = 8192, 4096, 4096
NCORES = 8
M_SHARD = M // NCORES          # 1024 rows of x per core
M_TILES = M_SHARD // P         # 8
K_SUBS = K // P                # 32
K_SUPERS = K // (2 * P)        # 16 (DoubleRow consumes 256 rows of K)
KH = K // 2                    # reduce half-width
N_BLK = 512
N_BLKS = N // N_BLK            # 8
NBG = N_BLKS // 2              # 4 groups of 2 weight slabs
TR_G = 8                       # transpose groups per tile (4 k-subtiles each)

FP8 = mybir.dt.float8e4
F32 = mybir.dt.float32
BF16 = mybir.dt.bfloat16

_PROGRAM_CACHE = {}


def _build_program():
    nc = bacc.Bacc(None, target_bir_lowering=False)

    x_d = nc.declare_dram_parameter("x", [M_SHARD, K], BF16, isOutput=False)
    # host layout: wt[nb, p, ksub, n] = weight[nb*512 + n, ksub*128 + p],
    # losslessly re-encoded to fp8 (reference weights are fp8-round-tripped)
    wt_d = nc.declare_dram_parameter("wt", [N_BLKS, P, K_SUBS, N_BLK], FP8, isOutput=False)
    ws_d = nc.declare_dram_parameter("wsb", [N], BF16, isOutput=False)
    bias_d = nc.declare_dram_parameter("biasb", [N], BF16, isOutput=False)
    out_d = nc.declare_dram_parameter("out", [M_SHARD, N], BF16, isOutput=True)

    x_ap = x_d[:]
    wt_ap = wt_d[:]
    out_ap = out_d[:]

    with tile.TileContext(nc) as tc, ExitStack() as ctx:
        singles = ctx.enter_context(tc.tile_pool(name="singles", bufs=1))
        xpool = ctx.enter_context(tc.tile_pool(name="xpool", bufs=3))
        stats = ctx.enter_context(tc.tile_pool(name="stats", bufs=4))
        xspool = ctx.enter_context(tc.tile_pool(name="xspool", bufs=M_TILES))
        diagpool = ctx.enter_context(tc.tile_pool(name="diagpool", bufs=2))
        xqtpool = ctx.enter_context(tc.tile_pool(name="xqtpool", bufs=M_TILES))
        wpool = ctx.enter_context(tc.tile_pool(name="wpool", bufs=4))
        opool = ctx.enter_context(tc.tile_pool(name="opool", bufs=4))
        psum_tr = ctx.enter_context(tc.tile_pool(name="psum_tr", bufs=4, space="PSUM"))
        psum_mm = ctx.enter_context(tc.tile_pool(name="psum_mm", bufs=4, space="PSUM"))

        # ---- prologue DMAs ----
        # x tiles stream on the sync ring, half-tiles so the amax reduce can
        # start before the full row block has landed
        xtiles = [None] * M_TILES

        def issue_x(mt):
            t = xpool.tile([P, K], BF16, tag="xt")
            nc.sync.dma_start(out=t[:, 0:KH], in_=x_ap[mt * P:(mt + 1) * P, 0:KH])
            nc.sync.dma_start(out=t[:, KH:K], in_=x_ap[mt * P:(mt + 1) * P, KH:K])
            xtiles[mt] = t

        issue_x(0)
        issue_x(1)

        # weight slabs on the ACT ring
        wslab_tiles = [None] * N_BLKS

        def issue_wslab(nb):
            t = wpool.tile([P, K_SUBS, N_BLK], FP8, tag="w")
            nc.scalar.dma_start(out=t[:], in_=wt_ap[nb])
            wslab_tiles[nb] = t

        issue_wslab(0)
        issue_wslab(1)

        # w-scale / bias broadcasts (bf16, host-prepped) ride the gpsimd ring
        ws_b = singles.tile([P, N], BF16)
        nc.gpsimd.dma_start(
            out=ws_b[:],
            in_=bass.AP(tensor=ws_d[:].tensor, offset=0, ap=[[0, P], [1, N]]),
        )
        bias_b = singles.tile([P, N], BF16)
        nc.gpsimd.dma_start(
            out=bias_b[:],
            in_=bass.AP(tensor=bias_d[:].tensor, offset=0, ap=[[0, P], [1, N]]),
        )

        ident = singles.tile([P, P], BF16)
        make_identity(nc, ident)

        xs_deq = [None] * M_TILES    # [P,1] f32 dequant scales, persistent
        xqt = [[None] * TR_G for _ in range(M_TILES)]   # fp8 x^T groups
        evict_rr = [0]               # round-robin counter for evict engines

        def quant_stats(mt):
            """amax reduce + scale chain + diag(inv) for tile mt."""
            xt = xtiles[mt]
            a1 = stats.tile([P, 1], F32, tag="a1")
            nc.vector.tensor_reduce(
                out=a1[:], in_=xt[:, 0:KH],
                axis=mybir.AxisListType.X, op=mybir.AluOpType.max,
                apply_absolute_value=True,
            )
            a2 = stats.tile([P, 1], F32, tag="a2")
            nc.vector.tensor_reduce(
                out=a2[:], in_=xt[:, KH:K],
                axis=mybir.AxisListType.X, op=mybir.AluOpType.max,
                apply_absolute_value=True,
            )
            with tc.high_priority():
                am = stats.tile([P, 1], F32, tag="am")
                nc.vector.tensor_tensor(
                    out=am[:], in0=a1[:], in1=a2[:], op=mybir.AluOpType.max)
                # u = max(amax, eps); r = 1/u; inv = bf16(224*r)
                u = stats.tile([P, 1], F32, tag="u")
                nc.vector.tensor_scalar(
                    out=u[:], in0=am[:], scalar1=1e-10, scalar2=None,
                    op0=mybir.AluOpType.max)
                r = stats.tile([P, 1], F32, tag="r")
                nc.vector.reciprocal(out=r[:], in_=u[:])
                invb = stats.tile([P, 1], BF16, tag="invb")
                nc.vector.tensor_scalar(
                    out=invb[:], in0=r[:], scalar1=224.0, scalar2=None,
                    op0=mybir.AluOpType.mult)
                invf = stats.tile([P, 1], F32, tag="invf")
                nc.vector.tensor_copy(out=invf[:], in_=invb[:])
                # dequant scale = exact reciprocal of the bf16 inv we apply
                xd = xspool.tile([P, 1], F32, tag="xs")
                nc.vector.reciprocal(out=xd[:], in_=invf[:])
                xs_deq[mt] = xd
                # diag(inv) in bf16: ident * inv (ACT per-partition scale)
                dg = diagpool.tile([P, P], BF16, tag="diag")
                nc.scalar.activation(
                    out=dg[:], in_=ident[:],
                    func=mybir.ActivationFunctionType.Copy, scale=invf[:])
            return dg

        def transpose_group(mt, g, dg):
            """Transpose 4 k-subtiles of tile mt through PE with diag scale."""
            ptr = psum_tr.tile([P, 4, P], F32, tag="ptr")
            for i in range(4):
                ks = 4 * g + i
                nc.tensor.matmul(
                    out=ptr[:, i, :],
                    lhsT=xtiles[mt][:, ks * P:(ks + 1) * P],
                    rhs=dg[:],
                    start=True, stop=True,
                )
            xq = xqtpool.tile([P, 4, P], FP8, tag=f"xqt{g}")
            # evictions alternate ACT(5/8) / DVE(3/8) to keep pace with PE
            k = evict_rr[0]
            evict_rr[0] += 1
            if k % 8 in (1, 4, 6):
                nc.vector.tensor_copy(out=xq[:], in_=ptr[:])
            else:
                nc.scalar.copy(out=xq[:], in_=ptr[:])
            xqt[mt][g] = xq

        def gemm_half(nbg, mt, bi, dg_next, next_mt):
            """16 DR matmuls for one 512-col block; optionally interleave 4
            transpose groups of the next tile ahead of them."""
            if dg_next is not None:
                for g in range(4 * bi, 4 * bi + 4):
                    transpose_group(next_mt, g, dg_next)
            nb = 2 * nbg + bi
            wslab = wslab_tiles[nb]
            pm = psum_mm.tile([P, N_BLK], F32, tag="pm")
            for j in range(K_SUPERS):
                off = 2 * (j % 2)
                nc.tensor.matmul(
                    out=pm[:],
                    lhsT=xqt[mt][j // 2][:, off:off + 2, :],
                    rhs=wslab[:, 2 * j:2 * j + 2, :],
                    start=(j == 0), stop=(j == K_SUPERS - 1),
                    perf_mode=mybir.MatmulPerfMode.DoubleRow,
                )
            return pm

        def out_stage(nbg, mt, pms):
            sb1 = opool.tile([P, 2 * N_BLK], BF16, tag="sb1")
            for bi, pm in enumerate(pms):
                nc.scalar.activation(
                    out=sb1[:, bi * N_BLK:(bi + 1) * N_BLK], in_=pm[:],
                    func=mybir.ActivationFunctionType.Copy,
                    scale=xs_deq[mt][:])
            sb2 = opool.tile([P, 2 * N_BLK], BF16, tag="sb2")
            c0, c1 = 2 * nbg * N_BLK, 2 * (nbg + 1) * N_BLK
            eng = nc.vector if mt % 2 == 0 else nc.gpsimd
            eng.tensor_mul(sb2[:], sb1[:], ws_b[:, c0:c1])
            eng.tensor_add(sb2[:], sb2[:], bias_b[:, c0:c1])
            nc.sync.dma_start(
                out=out_ap[mt * P:(mt + 1) * P, c0:c1], in_=sb2[:])

        # ---- main loop: nb-pair outer, m-tile inner ----
        dg = quant_stats(0)
        for g in range(TR_G):
            transpose_group(0, g, dg)

        for nbg in range(NBG):
            for mt in range(M_TILES):
                dg_next = None
                next_mt = mt + 1
                if nbg == 0 and next_mt < M_TILES:
                    if next_mt + 1 < M_TILES:
                        issue_x(next_mt + 1)
                    dg_next = quant_stats(next_mt)
                if mt == 0 and nbg + 1 < NBG:
                    issue_wslab(2 * nbg + 2)
                    issue_wslab(2 * nbg + 3)
                pmA = gemm_half(nbg, mt, 0, dg_next, next_mt)
                pmB = gemm_half(nbg, mt, 1, dg_next, next_mt)
                out_stage(nbg, mt, (pmA, pmB))

    nc.compile()
    return nc


def _get_program():
    if "nc" not in _PROGRAM_CACHE:
        _PROGRAM_CACHE["nc"] = _build_program()
    return _PROGRAM_CACHE["nc"]


def _run_sharded(x, weight, weight_scales, bias, trace=False):
    x = np.asarray(x).astype(ml_dtypes.bfloat16, copy=False)
    weight = np.asarray(weight, dtype=np.float32)
    weight_scales = np.asarray(weight_scales, dtype=np.float32)
    bias = np.asarray(bias, dtype=np.float32)

    # host-side sharding / layout only:
    # wt[nb, p, ksub, n] = weight[nb*512 + n, ksub*128 + p], re-encoded to
    # fp8 e4m3 (lossless: the reference weights are fp8-round-tripped values)
    wt = np.ascontiguousarray(
        weight.T.reshape(K_SUBS, P, N_BLKS, N_BLK).transpose(2, 1, 0, 3)
    ).astype(ml_dtypes.float8_e4m3)
    wsb = weight_scales.astype(ml_dtypes.bfloat16)
    biasb = bias.astype(ml_dtypes.bfloat16)
    in_maps = []
    for c in range(NCORES):
        in_maps.append({
            "x": np.ascontiguousarray(x[c * M_SHARD:(c + 1) * M_SHARD]),
            "wt": wt,
            "wsb": wsb,
            "biasb": biasb,
        })

    nc = _get_program()
    res = run_bass_kernel_spmd(nc, in_maps, core_ids=list(range(NCORES)), trace=trace)
    out = np.concatenate([res.results[c]["out"] for c in range(NCORES)], axis=0)
    return out, res.exec_time_ns


def kernel(x, weight, weight_scales, bias):
    out, _ = _run_sharded(x, weight, weight_scales, bias,
                          trace=bool(os.environ.get("KERNEL_TRACE")))
    return out


# revision 5
# speedup vs baseline: 1.0483x; 1.0483x over previous
"""Fp8 per-token/per-channel quantized linear for Trainium2, 8 NeuronCores.

Computation (matches the jax reference):
    amax[m]  = max_k |x[m, k]|                       (x is bf16)
    xs[m]    = max(amax, 1e-10) / 448
    x_q      = e4m3fn_round(x / xs)                  (values up to +-448)
    out      = bf16((x_q @ W^T) * xs * w_scales) + bf16(bias)

Mapping to TRN2 hardware:
  * TRN's fp8 E4M3 saturates at +-240 (256..448 are Inf/NaN), so we quantize
    at HALF scale: x_q' = e4m3_round(x * inv) with inv = bf16(224/amax), and
    dequantize with 1/inv.  The reference weights are already exactly
    fp8-representable, so casting them to fp8 is lossless.
  * Sharding: row-parallel over M (8 cores x 1024 rows).  Each core quantizes
    only its own rows and streams the full weight (fp8, host-transposed).
  * Quantization is FUSED into the on-chip transpose: the PE transposes each
    bf16 x tile with rhs = diag(inv) (so the matmul computes x^T * inv), and
    the PSUM->SBUF eviction casts f32->fp8.  No separate quant pass.
  * All bulk DMAs (x tiles, weight slabs, outputs) ride ONE queue (sync
    ring) in hand-ordered FIFO: the sync sequencer issues nothing else, so
    ring-credit stalls are harmless, and FIFO position = priority.
  * Loop order interleaves tile t+1's transposes between tile t's GEMM
    blocks so the in-order PE queue never phases on the quant pipeline.
  * Main GEMM runs in fp8 with perf_mode=DoubleRow (k=256 per matmul).
"""

import os
import numpy as np
import ml_dtypes
from contextlib import ExitStack

import concourse.bass as bass
import concourse.bacc as bacc
import concourse.tile as tile
from concourse import mybir
from concourse.bass_utils import run_bass_kernel_spmd
from concourse.masks import make_identity

P = 128
M, K, N = 8192, 4096, 4096
NCORES = 8
M_SHARD = M // NCORES          # 1024 rows of x per core
M_TILES = M_SHARD // P         # 8
K_SUBS = 32                    # 128-row k subtiles
K_SUPERS = 16                  # DoubleRow consumes 256 rows of K per matmul
KH = K // 2                    # x-tile DMA half / reduce half width
W_HSUB = K_SUBS // 2           # k-subtiles per weight half-slab
N_BLK = 512
N_BLKS = 8
NBG = N_BLKS // 2              # 4 groups of 2 weight slabs
TR_G = 8                       # transpose groups per tile (4 k-subtiles each)

FP8 = mybir.dt.float8e4
F32 = mybir.dt.float32
BF16 = mybir.dt.bfloat16

_PROGRAM_CACHE = {}


def _build_program():
    nc = bacc.Bacc(None, target_bir_lowering=False)

    x_d = nc.declare_dram_parameter("x", [M_SHARD, K], BF16, isOutput=False)
    # host layout: wt[nb, p, ksub, n] = weight[nb*512 + n, ksub*128 + p],
    # losslessly re-encoded to fp8 (reference weights are fp8-round-tripped)
    wt_d = nc.declare_dram_parameter("wt", [N_BLKS, P, K_SUBS, N_BLK], FP8, isOutput=False)
    ws_d = nc.declare_dram_parameter("wsb", [N], BF16, isOutput=False)
    bias_d = nc.declare_dram_parameter("biasb", [N], BF16, isOutput=False)
    out_d = nc.declare_dram_parameter("out", [M_SHARD, N], BF16, isOutput=True)

    x_ap = x_d[:]
    wt_ap = wt_d[:]
    out_ap = out_d[:]

    with tile.TileContext(nc) as tc, ExitStack() as ctx:
        singles = ctx.enter_context(tc.tile_pool(name="singles", bufs=1))
        xpool = ctx.enter_context(tc.tile_pool(name="xpool", bufs=3))
        stats = ctx.enter_context(tc.tile_pool(name="stats", bufs=4))
        xspool = ctx.enter_context(tc.tile_pool(name="xspool", bufs=M_TILES))
        diagpool = ctx.enter_context(tc.tile_pool(name="diagpool", bufs=2))
        xqtpool = ctx.enter_context(tc.tile_pool(name="xqtpool", bufs=M_TILES))
        wpool = ctx.enter_context(tc.tile_pool(name="wpool", bufs=8))
        opool = ctx.enter_context(tc.tile_pool(name="opool", bufs=4))
        psum_tr = ctx.enter_context(tc.tile_pool(name="psum_tr", bufs=4, space="PSUM"))
        psum_mm = ctx.enter_context(tc.tile_pool(name="psum_mm", bufs=4, space="PSUM"))

        # ---- bulk DMA issue helpers (all on the sync queue, FIFO-ordered) --
        xtiles = [None] * M_TILES

        def issue_x(mt):
            t = xpool.tile([P, K], BF16, tag="xt")
            nc.sync.dma_start(out=t[:, 0:KH], in_=x_ap[mt * P:(mt + 1) * P, 0:KH])
            nc.sync.dma_start(out=t[:, KH:K], in_=x_ap[mt * P:(mt + 1) * P, KH:K])
            xtiles[mt] = t

        # each weight slab = 2 half tiles of [P, 16, N_BLK] for finer deps
        whalves = [[None, None] for _ in range(N_BLKS)]

        def issue_wslab(nb):
            for h in range(2):
                t = wpool.tile([P, W_HSUB, N_BLK], FP8, tag="w")
                nc.sync.dma_start(
                    out=t[:], in_=wt_ap[nb, :, h * W_HSUB:(h + 1) * W_HSUB, :])
                whalves[nb][h] = t

        # prologue FIFO: x0, w0, x1, w1
        issue_x(0)
        issue_wslab(0)
        issue_x(1)
        issue_wslab(1)

        ident = singles.tile([P, P], BF16)
        make_identity(nc, ident)

        # w-scale / bias broadcasts (bf16, host-prepped) on the gpsimd queue,
        # issued a beat later so they don't contend with x0/w0
        ws_b = singles.tile([P, N], BF16)
        bias_b = singles.tile([P, N], BF16)

        def issue_broadcasts():
            nc.gpsimd.dma_start(
                out=ws_b[:],
                in_=bass.AP(tensor=ws_d[:].tensor, offset=0, ap=[[0, P], [1, N]]),
            )
            nc.gpsimd.dma_start(
                out=bias_b[:],
                in_=bass.AP(tensor=bias_d[:].tensor, offset=0, ap=[[0, P], [1, N]]),
            )

        xs_deq = [None] * M_TILES    # [P,1] f32 dequant scales, persistent
        xqt = [[None] * TR_G for _ in range(M_TILES)]   # fp8 x^T groups
        evict_rr = [0]               # round-robin counter for evict engines

        def quant_stats(mt):
            """amax reduce + scale chain + diag(inv) for tile mt."""
            xt = xtiles[mt]
            a1 = stats.tile([P, 1], F32, tag="a1")
            nc.vector.tensor_reduce(
                out=a1[:], in_=xt[:, 0:KH],
                axis=mybir.AxisListType.X, op=mybir.AluOpType.max,
                apply_absolute_value=True,
            )
            a2 = stats.tile([P, 1], F32, tag="a2")
            nc.vector.tensor_reduce(
                out=a2[:], in_=xt[:, KH:K],
                axis=mybir.AxisListType.X, op=mybir.AluOpType.max,
                apply_absolute_value=True,
            )
            with tc.high_priority():
                am = stats.tile([P, 1], F32, tag="am")
                nc.vector.tensor_tensor(
                    out=am[:], in0=a1[:], in1=a2[:], op=mybir.AluOpType.max)
                # u = max(amax, eps); r = 1/u; inv = bf16(224*r)
                u = stats.tile([P, 1], F32, tag="u")
                nc.vector.tensor_scalar(
                    out=u[:], in0=am[:], scalar1=1e-10, scalar2=None,
                    op0=mybir.AluOpType.max)
                r = stats.tile([P, 1], F32, tag="r")
                nc.vector.reciprocal(out=r[:], in_=u[:])
                invb = stats.tile([P, 1], BF16, tag="invb")
                nc.vector.tensor_scalar(
                    out=invb[:], in0=r[:], scalar1=224.0, scalar2=None,
                    op0=mybir.AluOpType.mult)
                invf = stats.tile([P, 1], F32, tag="invf")
                nc.vector.tensor_copy(out=invf[:], in_=invb[:])
                # dequant scale = exact reciprocal of the bf16 inv we apply
                xd = xspool.tile([P, 1], F32, tag="xs")
                nc.vector.reciprocal(out=xd[:], in_=invf[:])
                xs_deq[mt] = xd
                # diag(inv) in bf16: ident * inv (ACT per-partition scale)
                dg = diagpool.tile([P, P], BF16, tag="diag")
                nc.scalar.activation(
                    out=dg[:], in_=ident[:],
                    func=mybir.ActivationFunctionType.Copy, scale=invf[:])
            return dg

        def transpose_group(mt, g, dg):
            """Transpose 4 k-subtiles of tile mt through PE with diag scale."""
            ptr = psum_tr.tile([P, 4, P], F32, tag="ptr")
            for i in range(4):
                ks = 4 * g + i
                nc.tensor.matmul(
                    out=ptr[:, i, :],
                    lhsT=xtiles[mt][:, ks * P:(ks + 1) * P],
                    rhs=dg[:],
                    start=True, stop=True,
                )
            xq = xqtpool.tile([P, 4, P], FP8, tag=f"xqt{g}")
            # evictions alternate ACT(5/8) / DVE(3/8) to keep pace with PE
            k = evict_rr[0]
            evict_rr[0] += 1
            if k % 8 in (1, 4, 6):
                nc.vector.tensor_copy(out=xq[:], in_=ptr[:])
            else:
                nc.scalar.copy(out=xq[:], in_=ptr[:])
            xqt[mt][g] = xq

        def gemm_half(nbg, mt, bi):
            """16 DR matmuls for one 512-col output block."""
            nb = 2 * nbg + bi
            pm = psum_mm.tile([P, N_BLK], F32, tag="pm")
            for j in range(K_SUPERS):
                off = 2 * (j % 2)
                wh = whalves[nb][j // 8]
                jj = j % 8
                nc.tensor.matmul(
                    out=pm[:],
                    lhsT=xqt[mt][j // 2][:, off:off + 2, :],
                    rhs=wh[:, 2 * jj:2 * jj + 2, :],
                    start=(j == 0), stop=(j == K_SUPERS - 1),
                    perf_mode=mybir.MatmulPerfMode.DoubleRow,
                )
            return pm

        def out_stage(nbg, mt, pms):
            sb1 = opool.tile([P, 2 * N_BLK], BF16, tag="sb1")
            for bi, pm in enumerate(pms):
                nc.scalar.activation(
                    out=sb1[:, bi * N_BLK:(bi + 1) * N_BLK], in_=pm[:],
                    func=mybir.ActivationFunctionType.Copy,
                    scale=xs_deq[mt][:])
            sb2 = opool.tile([P, 2 * N_BLK], BF16, tag="sb2")
            c0, c1 = 2 * nbg * N_BLK, 2 * (nbg + 1) * N_BLK
            # gpsimd helps only during the quant phase (nbg 0); DVE has
            # plenty of slack afterwards and is much faster near the tail
            eng = nc.gpsimd if (nbg == 0 and mt % 2 == 1) else nc.vector
            eng.tensor_mul(sb2[:], sb1[:], ws_b[:, c0:c1])
            eng.tensor_add(sb2[:], sb2[:], bias_b[:, c0:c1])
            nc.sync.dma_start(
                out=out_ap[mt * P:(mt + 1) * P, c0:c1], in_=sb2[:])

        # ---- main loop: nb-pair outer, m-tile inner ----
        dg = quant_stats(0)
        for g in range(TR_G):
            transpose_group(0, g, dg)

        for nbg in range(NBG):
            for mt in range(M_TILES):
                dg_next = None
                next_mt = mt + 1
                if nbg == 0:
                    if next_mt + 1 < M_TILES:
                        issue_x(next_mt + 1)
                    if mt == 0:
                        issue_broadcasts()
                    if next_mt < M_TILES:
                        dg_next = quant_stats(next_mt)
                if mt == 4 and nbg + 1 < NBG:
                    issue_wslab(2 * nbg + 2)
                if mt == 6 and nbg + 1 < NBG:
                    issue_wslab(2 * nbg + 3)
                pmA = gemm_half(nbg, mt, 0)
                if dg_next is not None:
                    for g in range(4):
                        transpose_group(next_mt, g, dg_next)
                pmB = gemm_half(nbg, mt, 1)
                if dg_next is not None:
                    for g in range(4, TR_G):
                        transpose_group(next_mt, g, dg_next)
                out_stage(nbg, mt, (pmA, pmB))

    nc.compile()
    return nc


def _get_program():
    if "nc" not in _PROGRAM_CACHE:
        _PROGRAM_CACHE["nc"] = _build_program()
    return _PROGRAM_CACHE["nc"]


def _run_sharded(x, weight, weight_scales, bias, trace=False):
    x = np.asarray(x).astype(ml_dtypes.bfloat16, copy=False)
    weight = np.asarray(weight, dtype=np.float32)
    weight_scales = np.asarray(weight_scales, dtype=np.float32)
    bias = np.asarray(bias, dtype=np.float32)

    # host-side sharding / layout only:
    # wt[nb, p, ksub, n] = weight[nb*512 + n, ksub*128 + p], re-encoded to
    # fp8 e4m3 (lossless: the reference weights are fp8-round-tripped values)
    wt = np.ascontiguousarray(
        weight.T.reshape(K_SUBS, P, N_BLKS, N_BLK).transpose(2, 1, 0, 3)
    ).astype(ml_dtypes.float8_e4m3)
    wsb = weight_scales.astype(ml_dtypes.bfloat16)
    biasb = bias.astype(ml_dtypes.bfloat16)
    in_maps = []
    for c in range(NCORES):
        in_maps.append({
            "x": np.ascontiguousarray(x[c * M_SHARD:(c + 1) * M_SHARD]),
            "wt": wt,
            "wsb": wsb,
            "biasb": biasb,
        })

    nc = _get_program()
    res = run_bass_kernel_spmd(nc, in_maps, core_ids=list(range(NCORES)), trace=trace)
    out = np.concatenate([res.results[c]["out"] for c in range(NCORES)], axis=0)
    return out, res.exec_time_ns


def kernel(x, weight, weight_scales, bias):
    out, _ = _run_sharded(x, weight, weight_scales, bias,
                          trace=bool(os.environ.get("KERNEL_TRACE")))
    return out


# revision 7
# speedup vs baseline: 1.0851x; 1.0351x over previous
"""Fp8 per-token/per-channel quantized linear for Trainium2, 8 NeuronCores.

Computation (matches the jax reference):
    amax[m]  = max_k |x[m, k]|                       (x is bf16)
    xs[m]    = max(amax, 1e-10) / 448
    x_q      = e4m3fn_round(x / xs)                  (values up to +-448)
    out      = bf16((x_q @ W^T) * xs * w_scales) + bf16(bias)

Mapping to TRN2 hardware:
  * TRN's fp8 E4M3 saturates at +-240 (256..448 are Inf/NaN), so we quantize
    at HALF scale: x_q' = e4m3_round(x * inv) with inv = bf16(224/amax), and
    dequantize with 1/inv.  The reference weights are already exactly
    fp8-representable, so casting them to fp8 is lossless.
  * Sharding: row-parallel over M (8 cores x 1024 rows).  Each core quantizes
    only its own rows and streams the full weight (fp8, host-transposed).
  * Quantization is FUSED into the on-chip transpose: the PE transposes each
    bf16 x tile with rhs = diag(inv) (so the matmul computes x^T * inv), and
    the PSUM->SBUF eviction casts f32->fp8.  No separate quant pass.
  * All bulk DMAs (x tiles, weight slabs, outputs) ride ONE queue (sync
    ring) in hand-ordered FIFO: the sync sequencer issues nothing else, so
    ring-credit stalls are harmless, and FIFO position = priority.  The
    w-scale/bias broadcasts are per-nb-pair [128,1024] slices on the gpsimd
    queue so they never crowd out x at startup.
  * Loop order interleaves tile t+1's transposes between tile t's GEMM
    blocks so the in-order PE queue never phases on the quant pipeline.
  * Main GEMM runs in fp8 with perf_mode=DoubleRow (k=256 per matmul).
"""

import os
import numpy as np
import ml_dtypes
from contextlib import ExitStack

import concourse.bass as bass
import concourse.bacc as bacc
import concourse.tile as tile
from concourse import mybir
from concourse.bass_utils import run_bass_kernel_spmd
from concourse.masks import make_identity

P = 128
M, K, N = 8192, 4096, 4096
NCORES = 8
M_SHARD = M // NCORES          # 1024 rows of x per core
M_TILES = M_SHARD // P         # 8
K_SUBS = 32                    # 128-row k subtiles
K_SUPERS = 16                  # DoubleRow consumes 256 rows of K per matmul
KH = K // 2                    # x-tile DMA half / reduce half width
W_HSUB = K_SUBS // 2           # k-subtiles per weight half-slab
N_BLK = 512
N_BLKS = 8
NBG = N_BLKS // 2              # 4 groups of 2 weight slabs
NBW = 2 * N_BLK                # output columns per (nbg, mt) step
TR_G = 8                       # transpose groups per tile (4 k-subtiles each)

FP8 = mybir.dt.float8e4
F32 = mybir.dt.float32
BF16 = mybir.dt.bfloat16

_PROGRAM_CACHE = {}


def _build_program():
    nc = bacc.Bacc(None, target_bir_lowering=False)

    x_d = nc.declare_dram_parameter("x", [M_SHARD, K], BF16, isOutput=False)
    # host layout: wt[nb, p, ksub, n] = weight[nb*512 + n, ksub*128 + p],
    # losslessly re-encoded to fp8 (reference weights are fp8-round-tripped)
    wt_d = nc.declare_dram_parameter("wt", [N_BLKS, P, K_SUBS, N_BLK], FP8, isOutput=False)
    ws_d = nc.declare_dram_parameter("wsb", [N], BF16, isOutput=False)
    bias_d = nc.declare_dram_parameter("biasb", [N], BF16, isOutput=False)
    out_d = nc.declare_dram_parameter("out", [M_SHARD, N], BF16, isOutput=True)

    x_ap = x_d[:]
    wt_ap = wt_d[:]
    out_ap = out_d[:]

    with tile.TileContext(nc) as tc, ExitStack() as ctx:
        singles = ctx.enter_context(tc.tile_pool(name="singles", bufs=1))
        xpool = ctx.enter_context(tc.tile_pool(name="xpool", bufs=4))
        stats = ctx.enter_context(tc.tile_pool(name="stats", bufs=4))
        xspool = ctx.enter_context(tc.tile_pool(name="xspool", bufs=M_TILES))
        diagpool = ctx.enter_context(tc.tile_pool(name="diagpool", bufs=2))
        xqtpool = ctx.enter_context(tc.tile_pool(name="xqtpool", bufs=M_TILES))
        wpool = ctx.enter_context(tc.tile_pool(name="wpool", bufs=8))
        bcpool = ctx.enter_context(tc.tile_pool(name="bcpool", bufs=4))
        opool = ctx.enter_context(tc.tile_pool(name="opool", bufs=4))
        psum_tr = ctx.enter_context(tc.tile_pool(name="psum_tr", bufs=4, space="PSUM"))
        psum_mm = ctx.enter_context(tc.tile_pool(name="psum_mm", bufs=4, space="PSUM"))

        # ---- bulk DMA issue helpers (sync queue, FIFO-ordered by call) ----
        xtiles = [None] * M_TILES

        def issue_x(mt):
            t = xpool.tile([P, K], BF16, tag="xt")
            nc.sync.dma_start(out=t[:, 0:KH], in_=x_ap[mt * P:(mt + 1) * P, 0:KH])
            nc.sync.dma_start(out=t[:, KH:K], in_=x_ap[mt * P:(mt + 1) * P, KH:K])
            xtiles[mt] = t

        # each weight slab = 2 half tiles of [P, 16, N_BLK] for finer deps
        whalves = [[None, None] for _ in range(N_BLKS)]

        def issue_whalf(nb, h):
            t = wpool.tile([P, W_HSUB, N_BLK], FP8, tag="w")
            nc.sync.dma_start(
                out=t[:], in_=wt_ap[nb, :, h * W_HSUB:(h + 1) * W_HSUB, :])
            whalves[nb][h] = t

        # per-nbg [P, 1024] broadcast slices of w_scales / bias (gpsimd queue)
        ws_bc = [None] * NBG
        bias_bc = [None] * NBG

        def issue_bc(nbg):
            c0 = nbg * NBW
            t1 = bcpool.tile([P, NBW], BF16, tag="ws")
            nc.gpsimd.dma_start(
                out=t1[:],
                in_=bass.AP(tensor=ws_d[:].tensor, offset=c0, ap=[[0, P], [1, NBW]]),
            )
            ws_bc[nbg] = t1
            t2 = bcpool.tile([P, NBW], BF16, tag="bias")
            nc.gpsimd.dma_start(
                out=t2[:],
                in_=bass.AP(tensor=bias_d[:].tensor, offset=c0, ap=[[0, P], [1, NBW]]),
            )
            bias_bc[nbg] = t2

        # prologue FIFO: x0, x1, w0, x2, w1  (+ bc0 on the gpsimd queue)
        issue_x(0)
        issue_x(1)
        issue_whalf(0, 0)
        issue_whalf(0, 1)
        issue_x(2)
        issue_whalf(1, 0)
        issue_whalf(1, 1)
        issue_bc(0)

        ident = singles.tile([P, P], BF16)
        make_identity(nc, ident)
        # dependency-free ACT warmup so the lazy ACT_TABLE_LOAD happens now,
        # not in front of the first diag on the critical path
        warm = singles.tile([P, 1], BF16)
        nc.scalar.activation(
            out=warm[:], in_=ident[:, 0:1],
            func=mybir.ActivationFunctionType.Copy)

        xs_deq = [None] * M_TILES    # [P,1] f32 dequant scales, persistent
        xqt = [[None] * TR_G for _ in range(M_TILES)]   # fp8 x^T groups
        evict_rr = [0]               # round-robin counter for evict engines

        def quant_stats(mt):
            """amax reduce + scale chain + diag(inv) for tile mt."""
            xt = xtiles[mt]
            a1 = stats.tile([P, 1], F32, tag="a1")
            nc.vector.tensor_reduce(
                out=a1[:], in_=xt[:, 0:KH],
                axis=mybir.AxisListType.X, op=mybir.AluOpType.max,
                apply_absolute_value=True,
            )
            a2 = stats.tile([P, 1], F32, tag="a2")
            nc.vector.tensor_reduce(
                out=a2[:], in_=xt[:, KH:K],
                axis=mybir.AxisListType.X, op=mybir.AluOpType.max,
                apply_absolute_value=True,
            )
            with tc.high_priority():
                # u = max(a1, a2, eps) in one tensor_scalar (AP scalar + imm)
                u = stats.tile([P, 1], F32, tag="u")
                nc.vector.tensor_scalar(
                    out=u[:], in0=a1[:], scalar1=a2[:], scalar2=1e-10,
                    op0=mybir.AluOpType.max, op1=mybir.AluOpType.max)
                r = stats.tile([P, 1], F32, tag="r")
                nc.vector.reciprocal(out=r[:], in_=u[:])
                invb = stats.tile([P, 1], BF16, tag="invb")
                nc.vector.tensor_scalar(
                    out=invb[:], in0=r[:], scalar1=224.0, scalar2=None,
                    op0=mybir.AluOpType.mult)
                # f32 image of the bf16 inv (BIR: ACT scale AP must be fp32);
                # on ACT to keep the DVE chain short
                invf = stats.tile([P, 1], F32, tag="invf")
                nc.scalar.copy(out=invf[:], in_=invb[:])
                # dequant scale = exact reciprocal of the bf16 inv we apply
                xd = xspool.tile([P, 1], F32, tag="xs")
                nc.vector.reciprocal(out=xd[:], in_=invf[:])
                xs_deq[mt] = xd
                # diag(inv) in bf16: ident * inv (ACT per-partition scale)
                dg = diagpool.tile([P, P], BF16, tag="diag")
                nc.scalar.activation(
                    out=dg[:], in_=ident[:],
                    func=mybir.ActivationFunctionType.Copy, scale=invf[:])
            return dg

        def transpose_group(mt, g, dg):
            """Transpose 4 k-subtiles of tile mt through PE with diag scale."""
            ptr = psum_tr.tile([P, 4, P], F32, tag="ptr")
            for i in range(4):
                ks = 4 * g + i
                nc.tensor.matmul(
                    out=ptr[:, i, :],
                    lhsT=xtiles[mt][:, ks * P:(ks + 1) * P],
                    rhs=dg[:],
                    start=True, stop=True,
                )
            xq = xqtpool.tile([P, 4, P], FP8, tag=f"xqt{g}")
            # evictions alternate ACT(6/8) / DVE(2/8) to keep pace with PE
            k = evict_rr[0]
            evict_rr[0] += 1
            if k % 8 in (3, 6):
                nc.vector.tensor_copy(out=xq[:], in_=ptr[:])
            else:
                nc.scalar.copy(out=xq[:], in_=ptr[:])
            xqt[mt][g] = xq

        def gemm_half(nbg, mt, bi):
            """16 DR matmuls for one 512-col output block."""
            nb = 2 * nbg + bi
            pm = psum_mm.tile([P, N_BLK], F32, tag="pm")
            for j in range(K_SUPERS):
                off = 2 * (j % 2)
                wh = whalves[nb][j // 8]
                jj = j % 8
                nc.tensor.matmul(
                    out=pm[:],
                    lhsT=xqt[mt][j // 2][:, off:off + 2, :],
                    rhs=wh[:, 2 * jj:2 * jj + 2, :],
                    start=(j == 0), stop=(j == K_SUPERS - 1),
                    perf_mode=mybir.MatmulPerfMode.DoubleRow,
                )
            return pm

        def out_stage(nbg, mt, pms):
            sb1 = opool.tile([P, NBW], BF16, tag="sb1")
            for bi, pm in enumerate(pms):
                nc.scalar.activation(
                    out=sb1[:, bi * N_BLK:(bi + 1) * N_BLK], in_=pm[:],
                    func=mybir.ActivationFunctionType.Copy,
                    scale=xs_deq[mt][:])
            sb2 = opool.tile([P, NBW], BF16, tag="sb2")
            # quant phase (nbg 0): DVE is saturated, gpsimd is idle
            eng = nc.gpsimd if nbg == 0 else nc.vector
            eng.tensor_mul(sb2[:], sb1[:], ws_bc[nbg][:])
            eng.tensor_add(sb2[:], sb2[:], bias_bc[nbg][:])
            nc.sync.dma_start(
                out=out_ap[mt * P:(mt + 1) * P, nbg * NBW:(nbg + 1) * NBW],
                in_=sb2[:])

        # ---- main loop: nb-pair outer, m-tile inner ----
        dg = quant_stats(0)
        for g in range(TR_G):
            transpose_group(0, g, dg)

        for nbg in range(NBG):
            for mt in range(M_TILES):
                dg_next = None
                next_mt = mt + 1
                if nbg == 0:
                    if mt + 3 < M_TILES:
                        issue_x(mt + 3)
                    if 3 <= mt <= 6:
                        # w2/w3 half-slabs staggered through the quant phase
                        issue_whalf(2 + (mt - 3) // 2, (mt - 3) % 2)
                    if next_mt < M_TILES:
                        dg_next = quant_stats(next_mt)
                else:
                    if nbg + 1 < NBG and 2 <= mt <= 5:
                        issue_whalf(2 * (nbg + 1) + (mt - 2) // 2, mt % 2)
                if mt == 2 and nbg + 1 < NBG:
                    issue_bc(nbg + 1)
                pmA = gemm_half(nbg, mt, 0)
                if dg_next is not None:
                    for g in range(4):
                        transpose_group(next_mt, g, dg_next)
                pmB = gemm_half(nbg, mt, 1)
                if dg_next is not None:
                    for g in range(4, TR_G):
                        transpose_group(next_mt, g, dg_next)
                out_stage(nbg, mt, (pmA, pmB))

    nc.compile()
    return nc


def _get_program():
    if "nc" not in _PROGRAM_CACHE:
        _PROGRAM_CACHE["nc"] = _build_program()
    return _PROGRAM_CACHE["nc"]


def _run_sharded(x, weight, weight_scales, bias, trace=False):
    x = np.asarray(x).astype(ml_dtypes.bfloat16, copy=False)
    weight = np.asarray(weight, dtype=np.float32)
    weight_scales = np.asarray(weight_scales, dtype=np.float32)
    bias = np.asarray(bias, dtype=np.float32)

    # host-side sharding / layout only:
    # wt[nb, p, ksub, n] = weight[nb*512 + n, ksub*128 + p], re-encoded to
    # fp8 e4m3 (lossless: the reference weights are fp8-round-tripped values)
    wt = np.ascontiguousarray(
        weight.T.reshape(K_SUBS, P, N_BLKS, N_BLK).transpose(2, 1, 0, 3)
    ).astype(ml_dtypes.float8_e4m3)
    wsb = weight_scales.astype(ml_dtypes.bfloat16)
    biasb = bias.astype(ml_dtypes.bfloat16)
    in_maps = []
    for c in range(NCORES):
        in_maps.append({
            "x": np.ascontiguousarray(x[c * M_SHARD:(c + 1) * M_SHARD]),
            "wt": wt,
            "wsb": wsb,
            "biasb": biasb,
        })

    nc = _get_program()
    res = run_bass_kernel_spmd(nc, in_maps, core_ids=list(range(NCORES)), trace=trace)
    out = np.concatenate([res.results[c]["out"] for c in range(NCORES)], axis=0)
    return out, res.exec_time_ns


def kernel(x, weight, weight_scales, bias):
    out, _ = _run_sharded(x, weight, weight_scales, bias,
                          trace=bool(os.environ.get("KERNEL_TRACE")))
    return out


# revision 14
# speedup vs baseline: 1.0907x; 1.0052x over previous
"""Fp8 per-token/per-channel quantized linear for Trainium2, 8 NeuronCores.

Computation (matches the jax reference):
    amax[m]  = max_k |x[m, k]|                       (x is bf16)
    xs[m]    = max(amax, 1e-10) / 448
    x_q      = e4m3fn_round(x / xs)                  (values up to +-448)
    out      = bf16((x_q @ W^T) * xs * w_scales) + bf16(bias)

Mapping to TRN2 hardware:
  * TRN's fp8 E4M3 saturates at +-240 (256..448 are Inf/NaN), so we quantize
    at HALF scale: x_q' = e4m3_round(x * inv) with inv = bf16(224/amax), and
    dequantize with 1/inv.  The reference weights are already exactly
    fp8-representable, so casting them to fp8 is lossless.
  * Sharding: row-parallel over M (8 cores x 1024 rows).  Each core quantizes
    only its own rows and streams the full weight (fp8, host-transposed).
  * Quantization is FUSED into the on-chip transpose: the PE transposes each
    bf16 x tile with rhs = diag(inv) (so the matmul computes x^T * inv), and
    the PSUM->SBUF eviction casts f32->fp8.  No separate quant pass.
  * All bulk DMAs (x tiles, weight slabs, outputs) ride ONE queue (sync
    ring) in hand-ordered FIFO: the sync sequencer issues nothing else, so
    ring-credit stalls are harmless, and FIFO position = priority.  The
    w-scale/bias broadcasts are per-nb-pair [128,1024] slices on the gpsimd
    queue so they never crowd out x at startup.
  * Loop order interleaves tile t+1's transposes between tile t's GEMM
    blocks so the in-order PE queue never phases on the quant pipeline.
  * Main GEMM runs in fp8 with perf_mode=DoubleRow (k=256 per matmul).
"""

import os
import numpy as np
import ml_dtypes
from contextlib import ExitStack

import concourse.bass as bass
import concourse.bacc as bacc
import concourse.tile as tile
from concourse import mybir
from concourse.bass_utils import run_bass_kernel_spmd
from concourse.masks import make_identity

P = 128
M, K, N = 8192, 4096, 4096
NCORES = 8
M_SHARD = M // NCORES          # 1024 rows of x per core
M_TILES = M_SHARD // P         # 8
K_SUBS = 32                    # 128-row k subtiles
K_SUPERS = 16                  # DoubleRow consumes 256 rows of K per matmul
KH = K // 2                    # x-tile DMA half / reduce half width
W_HSUB = K_SUBS // 2           # k-subtiles per weight half-slab
N_BLK = 512
N_BLKS = 8
NBG = N_BLKS // 2              # 4 groups of 2 weight slabs
NBW = 2 * N_BLK                # output columns per (nbg, mt) step
TR_G = 8                       # transpose groups per tile (4 k-subtiles each)

FP8 = mybir.dt.float8e4
F32 = mybir.dt.float32
BF16 = mybir.dt.bfloat16

_PROGRAM_CACHE = {}


def _build_program():
    nc = bacc.Bacc(None, target_bir_lowering=False)

    x_d = nc.declare_dram_parameter("x", [M_SHARD, K], BF16, isOutput=False)
    # host layout: wt[nb, p, ksub, n] = weight[nb*512 + n, ksub*128 + p],
    # losslessly re-encoded to fp8 (reference weights are fp8-round-tripped)
    wt_d = nc.declare_dram_parameter("wt", [N_BLKS, P, K_SUBS, N_BLK], FP8, isOutput=False)
    # ws/bias pre-broadcast on host to [P, N] so the DMA is plain contiguous
    # rows (a stride-0 source AP lowers to per-element descriptors)
    ws_d = nc.declare_dram_parameter("wsb", [P, N], BF16, isOutput=False)
    bias_d = nc.declare_dram_parameter("biasb", [P, N], BF16, isOutput=False)
    out_d = nc.declare_dram_parameter("out", [M_SHARD, N], BF16, isOutput=True)

    x_ap = x_d[:]
    wt_ap = wt_d[:]
    out_ap = out_d[:]

    with tile.TileContext(nc) as tc, ExitStack() as ctx:
        singles = ctx.enter_context(tc.tile_pool(name="singles", bufs=1))
        xpool = ctx.enter_context(tc.tile_pool(name="xpool", bufs=4))
        stats = ctx.enter_context(tc.tile_pool(name="stats", bufs=4))
        xspool = ctx.enter_context(tc.tile_pool(name="xspool", bufs=M_TILES))
        diagpool = ctx.enter_context(tc.tile_pool(name="diagpool", bufs=2))
        xqtpool = ctx.enter_context(tc.tile_pool(name="xqtpool", bufs=M_TILES))
        wpool = ctx.enter_context(tc.tile_pool(name="wpool", bufs=8))
        bcpool = ctx.enter_context(tc.tile_pool(name="bcpool", bufs=4))
        opool = ctx.enter_context(tc.tile_pool(name="opool", bufs=4))
        psum_tr = ctx.enter_context(tc.tile_pool(name="psum_tr", bufs=4, space="PSUM"))
        psum_mm = ctx.enter_context(tc.tile_pool(name="psum_mm", bufs=4, space="PSUM"))

        # ---- bulk DMA issue helpers (sync queue, FIFO-ordered by call) ----
        xtiles = [None] * M_TILES

        def issue_x(mt):
            t = xpool.tile([P, K], BF16, tag="xt")
            nc.sync.dma_start(out=t[:, 0:KH], in_=x_ap[mt * P:(mt + 1) * P, 0:KH])
            nc.sync.dma_start(out=t[:, KH:K], in_=x_ap[mt * P:(mt + 1) * P, KH:K])
            xtiles[mt] = t

        # each weight slab = 2 half tiles of [P, 16, N_BLK] for finer deps
        whalves = [[None, None] for _ in range(N_BLKS)]

        def issue_whalf(nb, h):
            t = wpool.tile([P, W_HSUB, N_BLK], FP8, tag="w")
            nc.sync.dma_start(
                out=t[:], in_=wt_ap[nb, :, h * W_HSUB:(h + 1) * W_HSUB, :])
            whalves[nb][h] = t

        # per-nbg [P, 1024] broadcast slices of w_scales / bias (gpsimd queue)
        ws_bc = [None] * NBG
        bias_bc = [None] * NBG

        def issue_bc(nbg):
            c0 = nbg * NBW
            t1 = bcpool.tile([P, NBW], BF16, tag="ws")
            nc.gpsimd.dma_start(out=t1[:], in_=ws_d[:][:, c0:c0 + NBW])
            ws_bc[nbg] = t1
            t2 = bcpool.tile([P, NBW], BF16, tag="bias")
            nc.gpsimd.dma_start(out=t2[:], in_=bias_d[:][:, c0:c0 + NBW])
            bias_bc[nbg] = t2

        # prologue FIFO: x0, x1, w0, x2, w1  (+ bc0 on the gpsimd queue)
        issue_x(0)
        issue_x(1)
        issue_whalf(0, 0)
        issue_whalf(0, 1)
        issue_x(2)
        issue_whalf(1, 0)
        issue_whalf(1, 1)
        issue_bc(0)

        ident = singles.tile([P, P], BF16)
        make_identity(nc, ident)
        # dependency-free ACT warmup so the lazy ACT_TABLE_LOAD happens now,
        # not in front of the first diag on the critical path
        warm = singles.tile([P, 1], BF16)
        nc.scalar.activation(
            out=warm[:], in_=ident[:, 0:1],
            func=mybir.ActivationFunctionType.Copy)

        xs_deq = [None] * M_TILES    # [P,1] f32 dequant scales, persistent
        xqt = [[None] * TR_G for _ in range(M_TILES)]   # fp8 x^T groups
        evict_rr = [0]               # round-robin counter for evict engines
        chain_last = [None]          # last DVE chain inst of the prior tile
        dve_evicts = []              # recent DVE eviction insts

        def quant_stats(mt):
            """amax reduce + scale chain + diag(inv) for tile mt."""
            xt = xtiles[mt]
            a1 = stats.tile([P, 1], F32, tag="a1")
            red_a = nc.vector.tensor_reduce(
                out=a1[:], in_=xt[:, 0:KH],
                axis=mybir.AxisListType.X, op=mybir.AluOpType.max,
                apply_absolute_value=True,
            )
            a2 = stats.tile([P, 1], F32, tag="a2")
            red_b = nc.vector.tensor_reduce(
                out=a2[:], in_=xt[:, KH:K],
                axis=mybir.AxisListType.X, op=mybir.AluOpType.max,
                apply_absolute_value=True,
            )
            # DVE order hints: don't let these reduces jump ahead of the
            # previous tile's scale chain (gates diag -> PE transposes) or
            # of pending DVE psum evictions (gate the PE via psum_tr reuse)
            for red in (red_a, red_b):
                if chain_last[0] is not None:
                    tile.add_dep_helper(red.ins, chain_last[0].ins, sync=False,
                                        reason="chain before next reduce")
                for ev in dve_evicts:
                    tile.add_dep_helper(red.ins, ev.ins, sync=False,
                                        reason="evicts before next reduce")
            del dve_evicts[:]
            with tc.high_priority():
                # u = max(a1, a2, eps) in one tensor_scalar (AP scalar + imm)
                u = stats.tile([P, 1], F32, tag="u")
                nc.vector.tensor_scalar(
                    out=u[:], in0=a1[:], scalar1=a2[:], scalar2=1e-10,
                    op0=mybir.AluOpType.max, op1=mybir.AluOpType.max)
                r = stats.tile([P, 1], F32, tag="r")
                nc.vector.reciprocal(out=r[:], in_=u[:])
                invb = stats.tile([P, 1], BF16, tag="invb")
                nc.vector.tensor_scalar(
                    out=invb[:], in0=r[:], scalar1=224.0, scalar2=None,
                    op0=mybir.AluOpType.mult)
                # f32 image of the bf16 inv (BIR: ACT scale AP must be fp32);
                # on ACT to keep the DVE chain short
                invf = stats.tile([P, 1], F32, tag="invf")
                nc.scalar.copy(out=invf[:], in_=invb[:])
                # dequant scale = exact reciprocal of the bf16 inv we apply
                xd = xspool.tile([P, 1], F32, tag="xs")
                chain_last[0] = nc.vector.reciprocal(out=xd[:], in_=invf[:])
                xs_deq[mt] = xd
                # diag(inv) in bf16: ident * inv (ACT per-partition scale)
                dg = diagpool.tile([P, P], BF16, tag="diag")
                nc.scalar.activation(
                    out=dg[:], in_=ident[:],
                    func=mybir.ActivationFunctionType.Copy, scale=invf[:])
            return dg

        def transpose_group(mt, g, dg):
            """Transpose 4 k-subtiles of tile mt through PE with diag scale."""
            ptr = psum_tr.tile([P, 4, P], F32, tag="ptr")
            for i in range(4):
                ks = 4 * g + i
                nc.tensor.matmul(
                    out=ptr[:, i, :],
                    lhsT=xtiles[mt][:, ks * P:(ks + 1) * P],
                    rhs=dg[:],
                    start=True, stop=True,
                )
            xq = xqtpool.tile([P, 4, P], FP8, tag=f"xqt{g}")
            # evictions alternate ACT(6/8) / DVE(2/8) to keep pace with PE
            k = evict_rr[0]
            evict_rr[0] += 1
            if k % 8 in (3, 6):
                dve_evicts.append(nc.vector.tensor_copy(out=xq[:], in_=ptr[:]))
            else:
                nc.scalar.copy(out=xq[:], in_=ptr[:])
            xqt[mt][g] = xq

        def gemm_half(nbg, mt, bi):
            """16 DR matmuls for one 512-col output block."""
            nb = 2 * nbg + bi
            pm = psum_mm.tile([P, N_BLK], F32, tag="pm")
            for j in range(K_SUPERS):
                off = 2 * (j % 2)
                wh = whalves[nb][j // 8]
                jj = j % 8
                nc.tensor.matmul(
                    out=pm[:],
                    lhsT=xqt[mt][j // 2][:, off:off + 2, :],
                    rhs=wh[:, 2 * jj:2 * jj + 2, :],
                    start=(j == 0), stop=(j == K_SUPERS - 1),
                    perf_mode=mybir.MatmulPerfMode.DoubleRow,
                )
            return pm

        def out_stage(nbg, mt, pms):
            sb1 = opool.tile([P, NBW], BF16, tag="sb1")
            for bi, pm in enumerate(pms):
                nc.scalar.activation(
                    out=sb1[:, bi * N_BLK:(bi + 1) * N_BLK], in_=pm[:],
                    func=mybir.ActivationFunctionType.Copy,
                    scale=xs_deq[mt][:])
            sb2 = opool.tile([P, NBW], BF16, tag="sb2")
            # quant phase (nbg 0): DVE is saturated, gpsimd is idle
            eng = nc.gpsimd if nbg == 0 else nc.vector
            c0 = nbg * NBW
            if nbg == NBG - 1:
                # final phase: per-512 chains so the last DMA isn't one
                # serial 1MB tail behind the last matmul
                for bi in range(2):
                    s = slice(bi * N_BLK, (bi + 1) * N_BLK)
                    eng.tensor_mul(sb2[:, s], sb1[:, s], ws_bc[nbg][:, s])
                    eng.tensor_add(sb2[:, s], sb2[:, s], bias_bc[nbg][:, s])
                    nc.sync.dma_start(
                        out=out_ap[mt * P:(mt + 1) * P,
                                   c0 + bi * N_BLK:c0 + (bi + 1) * N_BLK],
                        in_=sb2[:, s])
            else:
                eng.tensor_mul(sb2[:], sb1[:], ws_bc[nbg][:])
                eng.tensor_add(sb2[:], sb2[:], bias_bc[nbg][:])
                nc.sync.dma_start(
                    out=out_ap[mt * P:(mt + 1) * P, c0:c0 + NBW], in_=sb2[:])

        # ---- main loop: nb-pair outer, m-tile inner ----
        dg = quant_stats(0)
        for g in range(TR_G):
            transpose_group(0, g, dg)

        for nbg in range(NBG):
            for mt in range(M_TILES):
                dg_next = None
                next_mt = mt + 1
                if nbg == 0:
                    if mt + 3 < M_TILES:
                        issue_x(mt + 3)
                    if 3 <= mt <= 6:
                        # w2/w3 half-slabs staggered through the quant phase
                        issue_whalf(2 + (mt - 3) // 2, (mt - 3) % 2)
                    if next_mt < M_TILES:
                        dg_next = quant_stats(next_mt)
                else:
                    if nbg + 1 < NBG and 2 <= mt <= 5:
                        issue_whalf(2 * (nbg + 1) + (mt - 2) // 2, mt % 2)
                if mt == 2 and nbg + 1 < NBG:
                    issue_bc(nbg + 1)
                pmA = gemm_half(nbg, mt, 0)
                if dg_next is not None:
                    for g in range(4):
                        transpose_group(next_mt, g, dg_next)
                pmB = gemm_half(nbg, mt, 1)
                if dg_next is not None:
                    for g in range(4, TR_G):
                        transpose_group(next_mt, g, dg_next)
                out_stage(nbg, mt, (pmA, pmB))

    nc.compile()
    return nc


def _get_program():
    if "nc" not in _PROGRAM_CACHE:
        _PROGRAM_CACHE["nc"] = _build_program()
    return _PROGRAM_CACHE["nc"]


def _run_sharded(x, weight, weight_scales, bias, trace=False):
    x = np.asarray(x).astype(ml_dtypes.bfloat16, copy=False)
    weight = np.asarray(weight, dtype=np.float32)
    weight_scales = np.asarray(weight_scales, dtype=np.float32)
    bias = np.asarray(bias, dtype=np.float32)

    # host-side sharding / layout only:
    # wt[nb, p, ksub, n] = weight[nb*512 + n, ksub*128 + p], re-encoded to
    # fp8 e4m3 (lossless: the reference weights are fp8-round-tripped values)
    wt = np.ascontiguousarray(
        weight.T.reshape(K_SUBS, P, N_BLKS, N_BLK).transpose(2, 1, 0, 3)
    ).astype(ml_dtypes.float8_e4m3)
    wsb = np.ascontiguousarray(
        np.broadcast_to(weight_scales.astype(ml_dtypes.bfloat16), (P, N)))
    biasb = np.ascontiguousarray(
        np.broadcast_to(bias.astype(ml_dtypes.bfloat16), (P, N)))
    in_maps = []
    for c in range(NCORES):
        in_maps.append({
            "x": np.ascontiguousarray(x[c * M_SHARD:(c + 1) * M_SHARD]),
            "wt": wt,
            "wsb": wsb,
            "biasb": biasb,
        })

    nc = _get_program()
    res = run_bass_kernel_spmd(nc, in_maps, core_ids=list(range(NCORES)), trace=trace)
    out = np.concatenate([res.results[c]["out"] for c in range(NCORES)], axis=0)
    return out, res.exec_time_ns


def kernel(x, weight, weight_scales, bias):
    out, _ = _run_sharded(x, weight, weight_scales, bias,
                          trace=bool(os.environ.get("KERNEL_TRACE")))
    return out


# revision 15
# speedup vs baseline: 1.0913x; 1.0005x over previous
"""Fp8 per-token/per-channel quantized linear for Trainium2, 8 NeuronCores.

Computation (matches the jax reference):
    amax[m]  = max_k |x[m, k]|                       (x is bf16)
    xs[m]    = max(amax, 1e-10) / 448
    x_q      = e4m3fn_round(x / xs)                  (values up to +-448)
    out      = bf16((x_q @ W^T) * xs * w_scales) + bf16(bias)

Mapping to TRN2 hardware:
  * TRN's fp8 E4M3 saturates at +-240 (256..448 are Inf/NaN), so we quantize
    at HALF scale: x_q' = e4m3_round(x * inv) with inv = bf16(224/amax), and
    dequantize with 1/inv.  The reference weights are already exactly
    fp8-representable, so casting them to fp8 is lossless.
  * Sharding: row-parallel over M (8 cores x 1024 rows).  Each core quantizes
    only its own rows and streams the full weight (fp8, host-transposed).
  * Quantization is FUSED into the on-chip transpose: the PE transposes each
    bf16 x tile with rhs = diag(inv) (so the matmul computes x^T * inv), and
    the ACT PSUM->SBUF eviction casts f32->fp8.  No separate quant pass.
  * DMA queues: each queue processes its DMAs serially with ~1.5us of fixed
    per-DMA overhead, so transfers are big (1-2MB) and split by purpose:
    x tiles + outputs on the sync queue (FIFO-priority-ordered), weight
    slabs on the ACT queue (its credits never block: issues are spaced),
    scale/bias broadcast slices on the gpsimd queue.
  * Loop order interleaves tile t+1's transposes between tile t's GEMM
    blocks so the in-order PE queue never phases on the quant pipeline;
    explicit order hints keep the next reduce behind the prior scale chain
    on the (reordering) DVE scheduler.
  * Main GEMM runs in fp8 with perf_mode=DoubleRow (k=256 per matmul).
"""

import os
import numpy as np
import ml_dtypes
from contextlib import ExitStack

import concourse.bass as bass
import concourse.bacc as bacc
import concourse.tile as tile
from concourse import mybir
from concourse.bass_utils import run_bass_kernel_spmd
from concourse.masks import make_identity

P = 128
M, K, N = 8192, 4096, 4096
NCORES = 8
M_SHARD = M // NCORES          # 1024 rows of x per core
M_TILES = M_SHARD // P         # 8
K_SUBS = 32                    # 128-row k subtiles
K_SUPERS = 16                  # DoubleRow consumes 256 rows of K per matmul
W_HSUB = K_SUBS // 2           # k-subtiles per weight half-slab
N_BLK = 512
N_BLKS = 8
NBG = N_BLKS // 2              # 4 groups of 2 weight slabs
NBW = 2 * N_BLK                # output columns per (nbg, mt) step
TR_G = 8                       # transpose groups per tile (4 k-subtiles each)

FP8 = mybir.dt.float8e4
F32 = mybir.dt.float32
BF16 = mybir.dt.bfloat16

_PROGRAM_CACHE = {}


def _build_program():
    nc = bacc.Bacc(None, target_bir_lowering=False)

    x_d = nc.declare_dram_parameter("x", [M_SHARD, K], BF16, isOutput=False)
    # host layout: wt[nb, p, ksub, n] = weight[nb*512 + n, ksub*128 + p],
    # losslessly re-encoded to fp8 (reference weights are fp8-round-tripped)
    wt_d = nc.declare_dram_parameter("wt", [N_BLKS, P, K_SUBS, N_BLK], FP8, isOutput=False)
    # ws/bias pre-broadcast on host to [P, N] so the DMA is plain contiguous
    # rows (a stride-0 source AP lowers to per-element descriptors)
    ws_d = nc.declare_dram_parameter("wsb", [P, N], BF16, isOutput=False)
    bias_d = nc.declare_dram_parameter("biasb", [P, N], BF16, isOutput=False)
    out_d = nc.declare_dram_parameter("out", [M_SHARD, N], BF16, isOutput=True)

    x_ap = x_d[:]
    wt_ap = wt_d[:]
    out_ap = out_d[:]

    with tile.TileContext(nc) as tc, ExitStack() as ctx:
        singles = ctx.enter_context(tc.tile_pool(name="singles", bufs=1))
        xpool = ctx.enter_context(tc.tile_pool(name="xpool", bufs=4))
        stats = ctx.enter_context(tc.tile_pool(name="stats", bufs=4))
        xspool = ctx.enter_context(tc.tile_pool(name="xspool", bufs=M_TILES))
        diagpool = ctx.enter_context(tc.tile_pool(name="diagpool", bufs=2))
        xqtpool = ctx.enter_context(tc.tile_pool(name="xqtpool", bufs=M_TILES))
        whpool = ctx.enter_context(tc.tile_pool(name="whpool", bufs=4))
        wfpool = ctx.enter_context(tc.tile_pool(name="wfpool", bufs=4))
        bcpool = ctx.enter_context(tc.tile_pool(name="bcpool", bufs=4))
        opool = ctx.enter_context(tc.tile_pool(name="opool", bufs=6))
        psum_tr = ctx.enter_context(tc.tile_pool(name="psum_tr", bufs=4, space="PSUM"))
        psum_mm = ctx.enter_context(tc.tile_pool(name="psum_mm", bufs=4, space="PSUM"))

        # ---- bulk DMA issue helpers ----
        # x tiles + outputs: sync queue (FIFO position = priority)
        xtiles = [None] * M_TILES

        def issue_x(mt):
            t = xpool.tile([P, K], BF16, tag="xt")
            nc.sync.dma_start(out=t[:], in_=x_ap[mt * P:(mt + 1) * P, :])
            xtiles[mt] = t

        # weight slabs: ACT queue. w0/w1 as halves (finer startup deps),
        # later slabs as single 2MB DMAs (less per-DMA overhead)
        whalves = [[None, None] for _ in range(2)]
        wfull = [None] * N_BLKS

        def issue_whalf(nb, h):
            t = whpool.tile([P, W_HSUB, N_BLK], FP8, tag="wh")
            nc.scalar.dma_start(
                out=t[:], in_=wt_ap[nb, :, h * W_HSUB:(h + 1) * W_HSUB, :])
            whalves[nb][h] = t

        def issue_wfull(nb):
            t = wfpool.tile([P, K_SUBS, N_BLK], FP8, tag="wf")
            nc.scalar.dma_start(out=t[:], in_=wt_ap[nb])
            wfull[nb] = t

        def w_rhs(nb, j):
            if nb < 2:
                wh = whalves[nb][j // 8]
                jj = j % 8
                return wh[:, 2 * jj:2 * jj + 2, :]
            return wfull[nb][:, 2 * j:2 * j + 2, :]

        # per-nbg [P, 1024] broadcast slices of w_scales / bias (gpsimd queue)
        ws_bc = [None] * NBG
        bias_bc = [None] * NBG

        def issue_bc(nbg):
            c0 = nbg * NBW
            t1 = bcpool.tile([P, NBW], BF16, tag="ws")
            nc.gpsimd.dma_start(out=t1[:], in_=ws_d[:][:, c0:c0 + NBW])
            ws_bc[nbg] = t1
            t2 = bcpool.tile([P, NBW], BF16, tag="bias")
            nc.gpsimd.dma_start(out=t2[:], in_=bias_d[:][:, c0:c0 + NBW])
            bias_bc[nbg] = t2

        # prologue
        issue_x(0)
        issue_x(1)
        issue_x(2)
        issue_whalf(0, 0)
        issue_whalf(0, 1)
        issue_whalf(1, 0)
        issue_whalf(1, 1)
        issue_bc(0)

        ident = singles.tile([P, P], BF16)
        make_identity(nc, ident)
        # dependency-free ACT warmup so the lazy ACT_TABLE_LOAD happens now,
        # not in front of the first diag on the critical path
        warm = singles.tile([P, 1], BF16)
        nc.scalar.activation(
            out=warm[:], in_=ident[:, 0:1],
            func=mybir.ActivationFunctionType.Copy)

        xs_deq = [None] * M_TILES    # [P,1] f32 dequant scales, persistent
        xqt = [[None] * TR_G for _ in range(M_TILES)]   # fp8 x^T groups
        chain_last = [None]          # last DVE chain inst of the prior tile

        def quant_stats(mt):
            """amax reduce + scale chain + diag(inv) for tile mt."""
            xt = xtiles[mt]
            am = stats.tile([P, 1], F32, tag="am")
            red = nc.vector.tensor_reduce(
                out=am[:], in_=xt[:],
                axis=mybir.AxisListType.X, op=mybir.AluOpType.max,
                apply_absolute_value=True,
            )
            # DVE order hint: don't let this reduce jump ahead of the
            # previous tile's scale chain (it gates diag -> PE transposes)
            if chain_last[0] is not None:
                tile.add_dep_helper(red.ins, chain_last[0].ins, sync=False,
                                    reason="chain before next reduce")
            with tc.high_priority():
                u = stats.tile([P, 1], F32, tag="u")
                nc.vector.tensor_scalar(
                    out=u[:], in0=am[:], scalar1=1e-10, scalar2=None,
                    op0=mybir.AluOpType.max)
                r = stats.tile([P, 1], F32, tag="r")
                nc.vector.reciprocal(out=r[:], in_=u[:])
                invb = stats.tile([P, 1], BF16, tag="invb")
                nc.vector.tensor_scalar(
                    out=invb[:], in0=r[:], scalar1=224.0, scalar2=None,
                    op0=mybir.AluOpType.mult)
                # f32 image of the bf16 inv (BIR: ACT scale AP must be fp32)
                invf = stats.tile([P, 1], F32, tag="invf")
                nc.scalar.copy(out=invf[:], in_=invb[:])
                # dequant scale = exact reciprocal of the bf16 inv we apply
                xd = xspool.tile([P, 1], F32, tag="xs")
                chain_last[0] = nc.vector.reciprocal(out=xd[:], in_=invf[:])
                xs_deq[mt] = xd
                # diag(inv) in bf16: ident * inv (ACT per-partition scale)
                dg = diagpool.tile([P, P], BF16, tag="diag")
                nc.scalar.activation(
                    out=dg[:], in_=ident[:],
                    func=mybir.ActivationFunctionType.Copy, scale=invf[:])
            return dg

        def transpose_group(mt, g, dg):
            """Transpose 4 k-subtiles of tile mt through PE with diag scale."""
            ptr = psum_tr.tile([P, 4, P], F32, tag="ptr")
            for i in range(4):
                ks = 4 * g + i
                nc.tensor.matmul(
                    out=ptr[:, i, :],
                    lhsT=xtiles[mt][:, ks * P:(ks + 1) * P],
                    rhs=dg[:],
                    start=True, stop=True,
                )
            xq = xqtpool.tile([P, 4, P], FP8, tag=f"xqt{g}")
            # all evictions on ACT: DVE stays clear for the reduce chain
            nc.scalar.copy(out=xq[:], in_=ptr[:])
            xqt[mt][g] = xq

        def gemm_half(nbg, mt, bi):
            """16 DR matmuls for one 512-col output block."""
            nb = 2 * nbg + bi
            pm = psum_mm.tile([P, N_BLK], F32, tag="pm")
            for j in range(K_SUPERS):
                off = 2 * (j % 2)
                nc.tensor.matmul(
                    out=pm[:],
                    lhsT=xqt[mt][j // 2][:, off:off + 2, :],
                    rhs=w_rhs(nb, j),
                    start=(j == 0), stop=(j == K_SUPERS - 1),
                    perf_mode=mybir.MatmulPerfMode.DoubleRow,
                )
            return pm

        def out_stage(nbg, mt, pms):
            sb1 = opool.tile([P, NBW], BF16, tag="sb1")
            for bi, pm in enumerate(pms):
                nc.scalar.activation(
                    out=sb1[:, bi * N_BLK:(bi + 1) * N_BLK], in_=pm[:],
                    func=mybir.ActivationFunctionType.Copy,
                    scale=xs_deq[mt][:])
            sb2 = opool.tile([P, NBW], BF16, tag="sb2")
            # quant phase (nbg 0): DVE is saturated, gpsimd is idle
            eng = nc.gpsimd if nbg == 0 else nc.vector
            c0 = nbg * NBW
            if nbg == NBG - 1:
                # final phase: per-512 chains so the last DMA isn't one
                # serial 1MB tail behind the last matmul
                for bi in range(2):
                    s = slice(bi * N_BLK, (bi + 1) * N_BLK)
                    eng.tensor_mul(sb2[:, s], sb1[:, s], ws_bc[nbg][:, s])
                    eng.tensor_add(sb2[:, s], sb2[:, s], bias_bc[nbg][:, s])
                    nc.sync.dma_start(
                        out=out_ap[mt * P:(mt + 1) * P,
                                   c0 + bi * N_BLK:c0 + (bi + 1) * N_BLK],
                        in_=sb2[:, s])
            else:
                eng.tensor_mul(sb2[:], sb1[:], ws_bc[nbg][:])
                eng.tensor_add(sb2[:], sb2[:], bias_bc[nbg][:])
                nc.sync.dma_start(
                    out=out_ap[mt * P:(mt + 1) * P, c0:c0 + NBW], in_=sb2[:])

        # ---- main loop: nb-pair outer, m-tile inner ----
        # weight slab prefetch spread to keep the phase-0 DMA load light:
        # (nbg, mt) -> slab to issue
        WSCHED = {(0, 6): 2, (1, 0): 3, (1, 2): 4, (1, 4): 5,
                  (2, 2): 6, (2, 4): 7}

        dg = quant_stats(0)
        for g in range(TR_G):
            transpose_group(0, g, dg)

        for nbg in range(NBG):
            for mt in range(M_TILES):
                dg_next = None
                next_mt = mt + 1
                if nbg == 0:
                    if mt + 3 < M_TILES:
                        issue_x(mt + 3)
                    if next_mt < M_TILES:
                        dg_next = quant_stats(next_mt)
                nb_pre = WSCHED.get((nbg, mt))
                if nb_pre is not None:
                    issue_wfull(nb_pre)
                if mt == 2 and nbg + 1 < NBG:
                    issue_bc(nbg + 1)
                pmA = gemm_half(nbg, mt, 0)
                if dg_next is not None:
                    for g in range(4):
                        transpose_group(next_mt, g, dg_next)
                pmB = gemm_half(nbg, mt, 1)
                if dg_next is not None:
                    for g in range(4, TR_G):
                        transpose_group(next_mt, g, dg_next)
                out_stage(nbg, mt, (pmA, pmB))

    nc.compile()
    return nc


def _get_program():
    if "nc" not in _PROGRAM_CACHE:
        _PROGRAM_CACHE["nc"] = _build_program()
    return _PROGRAM_CACHE["nc"]


def _run_sharded(x, weight, weight_scales, bias, trace=False):
    x = np.asarray(x).astype(ml_dtypes.bfloat16, copy=False)
    weight = np.asarray(weight, dtype=np.float32)
    weight_scales = np.asarray(weight_scales, dtype=np.float32)
    bias = np.asarray(bias, dtype=np.float32)

    # host-side sharding / layout only:
    # wt[nb, p, ksub, n] = weight[nb*512 + n, ksub*128 + p], re-encoded to
    # fp8 e4m3 (lossless: the reference weights are fp8-round-tripped values)
    wt = np.ascontiguousarray(
        weight.T.reshape(K_SUBS, P, N_BLKS, N_BLK).transpose(2, 1, 0, 3)
    ).astype(ml_dtypes.float8_e4m3)
    wsb = np.ascontiguousarray(
        np.broadcast_to(weight_scales.astype(ml_dtypes.bfloat16), (P, N)))
    biasb = np.ascontiguousarray(
        np.broadcast_to(bias.astype(ml_dtypes.bfloat16), (P, N)))
    in_maps = []
    for c in range(NCORES):
        in_maps.append({
            "x": np.ascontiguousarray(x[c * M_SHARD:(c + 1) * M_SHARD]),
            "wt": wt,
            "wsb": wsb,
            "biasb": biasb,
        })

    nc = _get_program()
    res = run_bass_kernel_spmd(nc, in_maps, core_ids=list(range(NCORES)), trace=trace)
    out = np.concatenate([res.results[c]["out"] for c in range(NCORES)], axis=0)
    return out, res.exec_time_ns


def kernel(x, weight, weight_scales, bias):
    out, _ = _run_sharded(x, weight, weight_scales, bias,
                          trace=bool(os.environ.get("KERNEL_TRACE")))
    return out


# revision 18
# speedup vs baseline: 1.1106x; 1.0176x over previous
"""Fp8 per-token/per-channel quantized linear for Trainium2, 8 NeuronCores.

Computation (matches the jax reference):
    amax[m]  = max_k |x[m, k]|                       (x is bf16)
    xs[m]    = max(amax, 1e-10) / 448
    x_q      = e4m3fn_round(x / xs)                  (values up to +-448)
    out      = bf16((x_q @ W^T) * xs * w_scales) + bf16(bias)

Mapping to TRN2 hardware:
  * TRN's fp8 E4M3 saturates at +-240 (256..448 are Inf/NaN), so we quantize
    at HALF scale: x_q' = e4m3_round(x * inv) with inv = bf16(224/amax), and
    dequantize with 1/inv.  The reference weights are already exactly
    fp8-representable, so casting them to fp8 is lossless.
  * Sharding: row-parallel over M (8 cores x 1024 rows).  Each core quantizes
    only its own rows and streams the full weight (fp8, host-transposed).
  * Quantization is FUSED into the on-chip transpose: the PE transposes each
    bf16 x tile with rhs = diag(inv) (so the matmul computes x^T * inv), and
    the ACT PSUM->SBUF eviction casts f32->fp8.  No separate quant pass.
  * DMA queues: each queue processes its DMAs serially with ~1.5us of fixed
    per-DMA overhead, so transfers are big (1-2MB) and split by purpose:
    x tiles + outputs on the sync queue (FIFO-priority-ordered), weight
    slabs on the ACT queue (its credits never block: issues are spaced),
    scale/bias broadcast slices on the gpsimd queue.
  * Loop order interleaves tile t+1's transposes between tile t's GEMM
    blocks so the in-order PE queue never phases on the quant pipeline;
    explicit order hints keep the next reduce behind the prior scale chain
    on the (reordering) DVE scheduler.
  * Main GEMM runs in fp8 with perf_mode=DoubleRow (k=256 per matmul).
"""

import os
import numpy as np
import ml_dtypes
from contextlib import ExitStack

import concourse.bass as bass
import concourse.bacc as bacc
import concourse.tile as tile
from concourse import mybir
from concourse.bass_utils import run_bass_kernel_spmd
from concourse.masks import make_identity

P = 128
M, K, N = 8192, 4096, 4096
NCORES = 8
M_SHARD = M // NCORES          # 1024 rows of x per core
M_TILES = M_SHARD // P         # 8
K_SUBS = 32                    # 128-row k subtiles
K_SUPERS = 16                  # DoubleRow consumes 256 rows of K per matmul
W_HSUB = K_SUBS // 2           # k-subtiles per weight half-slab
N_BLK = 512
N_BLKS = 8
NBG = N_BLKS // 2              # 4 groups of 2 weight slabs
NBW = 2 * N_BLK                # output columns per (nbg, mt) step
TR_G = 8                       # transpose groups per tile (4 k-subtiles each)

FP8 = mybir.dt.float8e4
F32 = mybir.dt.float32
BF16 = mybir.dt.bfloat16

_PROGRAM_CACHE = {}


def _build_program():
    nc = bacc.Bacc(None, target_bir_lowering=False)

    x_d = nc.declare_dram_parameter("x", [M_SHARD, K], BF16, isOutput=False)
    # host layout: wt[nb, p, ksub, n] = weight[nb*512 + n, ksub*128 + p],
    # losslessly re-encoded to fp8 (reference weights are fp8-round-tripped)
    wt_d = nc.declare_dram_parameter("wt", [N_BLKS, P, K_SUBS, N_BLK], FP8, isOutput=False)
    # ws/bias pre-broadcast on host to [P, N] so the DMA is plain contiguous
    # rows (a stride-0 source AP lowers to per-element descriptors)
    ws_d = nc.declare_dram_parameter("wsb", [P, N], BF16, isOutput=False)
    bias_d = nc.declare_dram_parameter("biasb", [P, N], BF16, isOutput=False)
    out_d = nc.declare_dram_parameter("out", [M_SHARD, N], BF16, isOutput=True)

    x_ap = x_d[:]
    wt_ap = wt_d[:]
    out_ap = out_d[:]

    with tile.TileContext(nc) as tc, ExitStack() as ctx:
        singles = ctx.enter_context(tc.tile_pool(name="singles", bufs=1))
        xpool = ctx.enter_context(tc.tile_pool(name="xpool", bufs=4))
        stats = ctx.enter_context(tc.tile_pool(name="stats", bufs=4))
        xspool = ctx.enter_context(tc.tile_pool(name="xspool", bufs=M_TILES))
        diagpool = ctx.enter_context(tc.tile_pool(name="diagpool", bufs=2))
        xqtpool = ctx.enter_context(tc.tile_pool(name="xqtpool", bufs=M_TILES))
        whpool = ctx.enter_context(tc.tile_pool(name="whpool", bufs=4))
        wfpool = ctx.enter_context(tc.tile_pool(name="wfpool", bufs=4))
        bcpool = ctx.enter_context(tc.tile_pool(name="bcpool", bufs=4))
        opool = ctx.enter_context(tc.tile_pool(name="opool", bufs=6))
        psum_tr = ctx.enter_context(tc.tile_pool(name="psum_tr", bufs=4, space="PSUM"))
        psum_mm = ctx.enter_context(tc.tile_pool(name="psum_mm", bufs=4, space="PSUM"))

        # ---- bulk DMA issue helpers ----
        # x tiles + outputs: sync queue (FIFO position = priority)
        xtiles = [None] * M_TILES

        def issue_x(mt):
            t = xpool.tile([P, K], BF16, tag="xt")
            nc.sync.dma_start(out=t[:], in_=x_ap[mt * P:(mt + 1) * P, :])
            xtiles[mt] = t

        # weight slabs: ACT queue. w0/w1 as halves (finer startup deps),
        # later slabs as single 2MB DMAs (less per-DMA overhead)
        whalves = [[None, None] for _ in range(2)]
        wfull = [None] * N_BLKS

        def issue_whalf(nb, h):
            t = whpool.tile([P, W_HSUB, N_BLK], FP8, tag="wh")
            nc.sync.dma_start(
                out=t[:], in_=wt_ap[nb, :, h * W_HSUB:(h + 1) * W_HSUB, :])
            whalves[nb][h] = t

        def issue_wfull(nb):
            t = wfpool.tile([P, K_SUBS, N_BLK], FP8, tag="wf")
            nc.sync.dma_start(out=t[:], in_=wt_ap[nb])
            wfull[nb] = t

        def w_rhs(nb, j):
            if nb < 2:
                wh = whalves[nb][j // 8]
                jj = j % 8
                return wh[:, 2 * jj:2 * jj + 2, :]
            return wfull[nb][:, 2 * j:2 * j + 2, :]

        # per-nbg [P, 1024] broadcast slices of w_scales / bias (gpsimd queue)
        ws_bc = [None] * NBG
        bias_bc = [None] * NBG

        def issue_bc(nbg):
            c0 = nbg * NBW
            t1 = bcpool.tile([P, NBW], BF16, tag="ws")
            nc.gpsimd.dma_start(out=t1[:], in_=ws_d[:][:, c0:c0 + NBW])
            ws_bc[nbg] = t1
            t2 = bcpool.tile([P, NBW], BF16, tag="bias")
            nc.gpsimd.dma_start(out=t2[:], in_=bias_d[:][:, c0:c0 + NBW])
            bias_bc[nbg] = t2

        # prologue (sync-queue FIFO: x0 first, then weights just-in-time)
        issue_x(0)
        issue_whalf(0, 0)
        issue_whalf(0, 1)
        issue_x(1)
        issue_whalf(1, 0)
        issue_whalf(1, 1)
        issue_x(2)
        issue_bc(0)

        ident = singles.tile([P, P], BF16)
        make_identity(nc, ident)
        # dependency-free ACT warmup so the lazy ACT_TABLE_LOAD happens now,
        # not in front of the first diag on the critical path
        warm = singles.tile([P, 1], BF16)
        nc.scalar.activation(
            out=warm[:], in_=ident[:, 0:1],
            func=mybir.ActivationFunctionType.Copy)

        xs_deq = [None] * M_TILES    # [P,1] f32 dequant scales, persistent
        xqt = [[None] * TR_G for _ in range(M_TILES)]   # fp8 x^T groups
        chain_last = [None]          # last DVE chain inst of the prior tile

        def quant_stats(mt):
            """amax reduce + scale chain + diag(inv) for tile mt."""
            xt = xtiles[mt]
            am = stats.tile([P, 1], F32, tag="am")
            red = nc.vector.tensor_reduce(
                out=am[:], in_=xt[:],
                axis=mybir.AxisListType.X, op=mybir.AluOpType.max,
                apply_absolute_value=True,
            )
            # DVE order hint: don't let this reduce jump ahead of the
            # previous tile's scale chain (it gates diag -> PE transposes)
            if chain_last[0] is not None:
                tile.add_dep_helper(red.ins, chain_last[0].ins, sync=False,
                                    reason="chain before next reduce")
            with tc.high_priority():
                u = stats.tile([P, 1], F32, tag="u")
                nc.vector.tensor_scalar(
                    out=u[:], in0=am[:], scalar1=1e-10, scalar2=None,
                    op0=mybir.AluOpType.max)
                r = stats.tile([P, 1], F32, tag="r")
                nc.vector.reciprocal(out=r[:], in_=u[:])
                invb = stats.tile([P, 1], BF16, tag="invb")
                nc.vector.tensor_scalar(
                    out=invb[:], in0=r[:], scalar1=224.0, scalar2=None,
                    op0=mybir.AluOpType.mult)
                # f32 image of the bf16 inv (BIR: ACT scale AP must be fp32)
                invf = stats.tile([P, 1], F32, tag="invf")
                nc.scalar.copy(out=invf[:], in_=invb[:])
                # dequant scale = exact reciprocal of the bf16 inv we apply
                xd = xspool.tile([P, 1], F32, tag="xs")
                chain_last[0] = nc.vector.reciprocal(out=xd[:], in_=invf[:])
                xs_deq[mt] = xd
                # diag(inv) in bf16: ident * inv (ACT per-partition scale)
                dg = diagpool.tile([P, P], BF16, tag="diag")
                nc.scalar.activation(
                    out=dg[:], in_=ident[:],
                    func=mybir.ActivationFunctionType.Copy, scale=invf[:])
            return dg

        def transpose_group(mt, g, dg):
            """Transpose 4 k-subtiles of tile mt through PE with diag scale."""
            ptr = psum_tr.tile([P, 4, P], F32, tag="ptr")
            for i in range(4):
                ks = 4 * g + i
                nc.tensor.matmul(
                    out=ptr[:, i, :],
                    lhsT=xtiles[mt][:, ks * P:(ks + 1) * P],
                    rhs=dg[:],
                    start=True, stop=True,
                )
            xq = xqtpool.tile([P, 4, P], FP8, tag=f"xqt{g}")
            # evictions on ACT (DVE stays clear for the reduce chain); for
            # tile 0 alternate with DVE — it's idle until x1 lands, and the
            # serial ACT evicts would otherwise pace the first GEMM block
            if mt == 0 and g % 2 == 1:
                nc.vector.tensor_copy(out=xq[:], in_=ptr[:])
            else:
                nc.scalar.copy(out=xq[:], in_=ptr[:])
            xqt[mt][g] = xq

        def gemm_half(nbg, mt, bi):
            """16 DR matmuls for one 512-col output block."""
            nb = 2 * nbg + bi
            pm = psum_mm.tile([P, N_BLK], F32, tag="pm")
            for j in range(K_SUPERS):
                off = 2 * (j % 2)
                nc.tensor.matmul(
                    out=pm[:],
                    lhsT=xqt[mt][j // 2][:, off:off + 2, :],
                    rhs=w_rhs(nb, j),
                    start=(j == 0), stop=(j == K_SUPERS - 1),
                    perf_mode=mybir.MatmulPerfMode.DoubleRow,
                )
            return pm

        def out_stage(nbg, mt, pms):
            sb1 = opool.tile([P, NBW], BF16, tag="sb1")
            for bi, pm in enumerate(pms):
                nc.scalar.activation(
                    out=sb1[:, bi * N_BLK:(bi + 1) * N_BLK], in_=pm[:],
                    func=mybir.ActivationFunctionType.Copy,
                    scale=xs_deq[mt][:])
            sb2 = opool.tile([P, NBW], BF16, tag="sb2")
            # quant phase (nbg 0): DVE is saturated, gpsimd is idle
            eng = nc.gpsimd if nbg == 0 else nc.vector
            c0 = nbg * NBW
            if nbg == NBG - 1:
                # final phase: per-512 chains so the last DMA isn't one
                # serial 1MB tail behind the last matmul
                for bi in range(2):
                    s = slice(bi * N_BLK, (bi + 1) * N_BLK)
                    eng.tensor_mul(sb2[:, s], sb1[:, s], ws_bc[nbg][:, s])
                    eng.tensor_add(sb2[:, s], sb2[:, s], bias_bc[nbg][:, s])
                    nc.sync.dma_start(
                        out=out_ap[mt * P:(mt + 1) * P,
                                   c0 + bi * N_BLK:c0 + (bi + 1) * N_BLK],
                        in_=sb2[:, s])
            else:
                eng.tensor_mul(sb2[:], sb1[:], ws_bc[nbg][:])
                eng.tensor_add(sb2[:], sb2[:], bias_bc[nbg][:])
                nc.sync.dma_start(
                    out=out_ap[mt * P:(mt + 1) * P, c0:c0 + NBW], in_=sb2[:])

        # ---- main loop: nb-pair outer, m-tile inner ----
        # weight slab prefetch spread to keep the phase-0 DMA load light:
        # (nbg, mt) -> slab to issue
        WSCHED = {(0, 6): 2, (1, 0): 3, (1, 2): 4, (1, 4): 5,
                  (2, 2): 6, (2, 4): 7}

        dg = quant_stats(0)
        for g in range(TR_G):
            transpose_group(0, g, dg)

        for nbg in range(NBG):
            for mt in range(M_TILES):
                dg_next = None
                next_mt = mt + 1
                if nbg == 0:
                    if mt + 3 < M_TILES:
                        issue_x(mt + 3)
                    if next_mt < M_TILES:
                        dg_next = quant_stats(next_mt)
                nb_pre = WSCHED.get((nbg, mt))
                if nb_pre is not None:
                    issue_wfull(nb_pre)
                if mt == 2 and nbg + 1 < NBG:
                    issue_bc(nbg + 1)
                pmA = gemm_half(nbg, mt, 0)
                if dg_next is not None:
                    for g in range(4):
                        transpose_group(next_mt, g, dg_next)
                pmB = gemm_half(nbg, mt, 1)
                if dg_next is not None:
                    for g in range(4, TR_G):
                        transpose_group(next_mt, g, dg_next)
                out_stage(nbg, mt, (pmA, pmB))

    nc.compile()
    return nc


def _get_program():
    if "nc" not in _PROGRAM_CACHE:
        _PROGRAM_CACHE["nc"] = _build_program()
    return _PROGRAM_CACHE["nc"]


def _run_sharded(x, weight, weight_scales, bias, trace=False):
    x = np.asarray(x).astype(ml_dtypes.bfloat16, copy=False)
    weight = np.asarray(weight, dtype=np.float32)
    weight_scales = np.asarray(weight_scales, dtype=np.float32)
    bias = np.asarray(bias, dtype=np.float32)

    # host-side sharding / layout only:
    # wt[nb, p, ksub, n] = weight[nb*512 + n, ksub*128 + p], re-encoded to
    # fp8 e4m3 (lossless: the reference weights are fp8-round-tripped values)
    wt = np.ascontiguousarray(
        weight.T.reshape(K_SUBS, P, N_BLKS, N_BLK).transpose(2, 1, 0, 3)
    ).astype(ml_dtypes.float8_e4m3)
    wsb = np.ascontiguousarray(
        np.broadcast_to(weight_scales.astype(ml_dtypes.bfloat16), (P, N)))
    biasb = np.ascontiguousarray(
        np.broadcast_to(bias.astype(ml_dtypes.bfloat16), (P, N)))
    in_maps = []
    for c in range(NCORES):
        in_maps.append({
            "x": np.ascontiguousarray(x[c * M_SHARD:(c + 1) * M_SHARD]),
            "wt": wt,
            "wsb": wsb,
            "biasb": biasb,
        })

    nc = _get_program()
    res = run_bass_kernel_spmd(nc, in_maps, core_ids=list(range(NCORES)), trace=trace)
    out = np.concatenate([res.results[c]["out"] for c in range(NCORES)], axis=0)
    return out, res.exec_time_ns


def kernel(x, weight, weight_scales, bias):
    out, _ = _run_sharded(x, weight, weight_scales, bias,
                          trace=bool(os.environ.get("KERNEL_TRACE")))
    return out
